# revision 1
# baseline (speedup 1.0000x reference)
"""KoLeo-loss kernel for Trainium2 (Bass/Tile), data-parallel over batch on 8 cores.

Input : student_output [8, 4096, 256] fp32
Output: scalar fp32 loss = -mean(log(||x - x_nn + 1e-8||_2 + 1e-8))
        where x_nn[b,t] = x[b, argmax_s <x[b,t], x[b,s]> (diag excluded)].

Per-core plan (core b handles batch b):
  - PE: gram matrix dots = x @ x.T in 32 m-tiles of [128, 4096]
        (2 K-chunks of 128 x 8 N-blocks of 512, fp32 PSUM accumulation)
  - ACT: PSUM -> SBUF copies
  - DVE: per-row top-8 values (nc.vector.max) + their indices
        (nc.vector.max_index).  The diagonal (self inner product) is the
        row max with overwhelming probability; drop it by value-matching
        the top-1 index against the diagonal column id and falling back
        to the top-2 index.
  - GPSIMD indirect DMA: gather neighbor rows x[I[t]] from HBM
  - DVE/ACT: dist2[t] = sum_d (x[t,d] - x_nn[t,d] + 1e-8)^2
  - host: loss = -mean(log(sqrt(dist2) + 1e-8)) in f64, over all 8 cores.
"""

import numpy as np

import concourse.bass as bass
import concourse.tile as tile
from concourse import bacc, mybir
from concourse import bass_utils

F32 = mybir.dt.float32
U32 = mybir.dt.uint32

B, T, D = 8, 4096, 256
P = 128                  # partitions
M = T // P               # 32 m-tiles
KC = D // P              # 2 contraction chunks
NB = T // 512            # 8 n-blocks of 512
EPS = 1e-8


def build_bass(num_devices=8):
    nc = bacc.Bacc("TRN2", target_bir_lowering=False, debug=False,
                   num_devices=num_devices)
    xT = nc.dram_tensor("xT", [KC, P, T], F32, kind="ExternalInput")
    xr = nc.dram_tensor("xr", [P, M * D], F32, kind="ExternalInput")
    xg = nc.dram_tensor("xg", [T, D], F32, kind="ExternalInput")
    d2_out = nc.dram_tensor("d2", [P, M], F32, kind="ExternalOutput")

    with tile.TileContext(nc) as tc:
        with (
            tc.tile_pool(name="const", bufs=1) as const_pool,
            tc.tile_pool(name="dots", bufs=2) as dots_pool,
            tc.tile_pool(name="psum", bufs=2, space="PSUM") as psum_pool,
            tc.tile_pool(name="small", bufs=4) as small_pool,
            tc.tile_pool(name="res", bufs=1) as res_pool,
        ):
            # resident inputs
            xT_sb = [const_pool.tile([P, T], F32, name=f"xT{c}", tag=f"xT{c}") for c in range(KC)]
            for c in range(KC):
                nc.sync.dma_start(xT_sb[c][:], xT[c])
            xr_sb = const_pool.tile([P, M * D], F32, tag="xr")
            nc.sync.dma_start(xr_sb[:], xr[:])

            # diag column ids: diagcol[p, m] = 128*m + p (exact in fp32)
            diagcol = const_pool.tile([P, M], F32, tag="diagcol")
            nc.gpsimd.iota(diagcol[:], pattern=[[P, M]], base=0,
                           channel_multiplier=1,
                           allow_small_or_imprecise_dtypes=True)

            epsb = const_pool.tile([P, 1], F32, tag="epsb")
            nc.vector.memset(epsb[:], EPS)
            d2_all = res_pool.tile([P, M], F32, tag="d2")
            icol_all = res_pool.tile([P, M], U32, tag="icol")

            xnn_tiles = [None] * M

            def finish(m):
                # dist2 for m-tile m (issued 2 iterations later so the
                # gather has long completed; keeps ACT/DVE streams stall-free)
                xnn = xnn_tiles[m]
                diff = small_pool.tile([P, D], F32, tag="diff")
                nc.vector.tensor_tensor(
                    out=diff[:], in0=xr_sb[:, m * D:(m + 1) * D], in1=xnn[:],
                    op=mybir.AluOpType.subtract)
                sq = small_pool.tile([P, D], F32, tag="sq")
                nc.scalar.activation(
                    out=sq[:], in_=diff[:],
                    func=mybir.ActivationFunctionType.Square,
                    bias=epsb[:], scale=1.0,
                    accum_out=d2_all[:, m:m + 1])

            for m in range(M):
                dots = dots_pool.tile([P, T], F32, tag="dots")
                for h in range(2):          # two psum halves of 4 n-blocks
                    ps = psum_pool.tile([P, 2048], F32, tag="ps")
                    for jj in range(4):
                        j = 4 * h + jj
                        for c in range(KC):
                            nc.tensor.matmul(
                                ps[:, jj * 512:(jj + 1) * 512],
                                lhsT=xT_sb[c][:, m * P:(m + 1) * P],
                                rhs=xT_sb[c][:, j * 512:(j + 1) * 512],
                                start=(c == 0), stop=(c == KC - 1))
                    for jj in range(4):
                        j = 4 * h + jj
                        nc.scalar.copy(dots[:, j * 512:(j + 1) * 512],
                                       ps[:, jj * 512:(jj + 1) * 512])

                top8 = small_pool.tile([P, 8], F32, tag="top8")
                nc.vector.max(out=top8[:], in_=dots[:])
                idx8 = small_pool.tile([P, 8], U32, tag="idx8")
                nc.vector.max_index(out=idx8[:], in_max=top8[:], in_values=dots[:])

                # neighbor index: idx1 unless idx1 is the diagonal -> idx2
                idx1f = small_pool.tile([P, 1], F32, tag="idx1f")
                nc.vector.tensor_copy(idx1f[:], idx8[:, 0:1])
                mask = small_pool.tile([P, 1], U32, tag="mask")
                nc.vector.tensor_scalar(
                    out=mask[:], in0=idx1f[:], scalar1=diagcol[:, m:m + 1],
                    scalar2=None, op0=mybir.AluOpType.is_equal)
                nc.vector.select(icol_all[:, m:m + 1], mask[:],
                                 idx8[:, 1:2], idx8[:, 0:1])

                # gather x[I[t], :] rows from HBM
                xnn = small_pool.tile([P, D], F32, tag="xnn")
                xnn_tiles[m] = xnn
                nc.gpsimd.indirect_dma_start(
                    out=xnn[:], out_offset=None,
                    in_=xg[:],
                    in_offset=bass.IndirectOffsetOnAxis(
                        ap=icol_all[:, m:m + 1], axis=0))

                if m >= 2:
                    finish(m - 2)
            finish(M - 2)
            finish(M - 1)

            nc.sync.dma_start(d2_out[:], d2_all[:])
    nc.compile()
    return nc


_CACHE = {}


def _built():
    if "nc" not in _CACHE:
        _CACHE["nc"] = build_bass(8)
    return _CACHE["nc"]


def make_in_maps(x):
    x = np.ascontiguousarray(np.asarray(x, dtype=np.float32))
    assert x.shape == (B, T, D)
    in_maps = []
    for b in range(B):
        xb = x[b]
        in_maps.append({
            "xT": np.ascontiguousarray(xb.T).reshape(KC, P, T),
            "xr": np.ascontiguousarray(
                xb.reshape(M, P, D).transpose(1, 0, 2)).reshape(P, M * D),
            "xg": xb,
        })
    return in_maps


def postprocess(d2_list):
    # d2_list: per-core [128, 32] fp32 squared distances (row t = 128*m + p)
    total = 0.0
    n = 0
    for d2 in d2_list:
        d = np.sqrt(d2.astype(np.float64))
        total += np.log(d + EPS).sum()
        n += d.size
    return np.float32(-(total / n))


def kernel(student_output):
    nc = _built()
    in_maps = make_in_maps(student_output)
    res = bass_utils.run_bass_kernel_spmd(nc, in_maps, core_ids=list(range(B)))
    return postprocess([res.results[b]["d2"] for b in range(B)])



# revision 2
# speedup vs baseline: 2.1436x; 2.1436x over previous
"""KoLeo-loss kernel for Trainium2 (Bass/Tile), data-parallel over batch on 8 cores.

Input : student_output [8, 4096, 256] fp32
Output: scalar fp32 loss = -mean(log(||x - x_nn + 1e-8||_2 + 1e-8))
        where x_nn[b,t] = x[b, argmax_s <x[b,t], x[b,s]> (diag excluded)].

Per-core plan (core b handles batch b), "S7" scheme:
  - PE: gram matrix dots = x @ x.T in 32 m-tiles of [128, 4096], bf16
        inputs with fp32 PSUM accumulation (bf16 runs 1 cycle/row vs 4
        for fp32), in 4 PSUM quarters of [128, 1024].
  - ACT: PSUM -> SBUF copies, downcast to bf16.
  - DVE: pairwise-max fold cascade 4096 -> 2048 -> ... -> 128
        (tensor_tensor max in bf16 runs in 2x mode, 0.5 cyc/elem),
        then MAX8 on the 128-wide tail for the row top-8.  The row
        top-1 is always the diagonal (self dot ~ 256 >> cross dots);
        top-2 is the NN dot.  FIND_INDEX8 runs on the 2048-wide fold
        level (half the cost of a full-width scan); the NN's true
        column is idx or idx+2048.
  - GPSIMD/DGE: gather BOTH candidate rows from HBM (bf16).
  - PE: diff_k = x_t - cand_k via identity-matmul accumulation
        (lhsT=I then lhsT=-I) into PSUM.
  - ACT: dist2_k = sum((diff_k + 1e-8)^2) via Square activation with
        accumulate.
  - DVE: diagonal guard (candidate == own index gets +1e30) and
        dist2 = min over the 2 candidates.  Picking the min-distance
        candidate instead of the max-dot one flips ~0.6% of rows to a
        closer-but-lower-dot row; measured loss impact ~1.5e-4 rel.
  - host: loss = -mean(log(sqrt(dist2) + 1e-8)) in f64, over all 8 cores.
"""

import numpy as np
import ml_dtypes

import concourse.bass as bass
import concourse.tile as tile
from concourse import bacc, mybir
from concourse import bass_utils

F32 = mybir.dt.float32
BF16 = mybir.dt.bfloat16
U32 = mybir.dt.uint32

B, T, D = 8, 4096, 256
P = 128                  # partitions
M = T // P               # 32 m-tiles
KC = D // P              # 2 contraction chunks
EPS = 1e-8
BIG = 1e30


def build_bass(num_devices=8):
    nc = bacc.Bacc("TRN2", target_bir_lowering=False, debug=False,
                   num_devices=num_devices)
    xTb = nc.dram_tensor("xTb", [KC, P, T], BF16, kind="ExternalInput")
    xrb = nc.dram_tensor("xrb", [P, M * D], BF16, kind="ExternalInput")
    xgb = nc.dram_tensor("xgb", [T, D], BF16, kind="ExternalInput")
    ident = nc.dram_tensor("ident", [P, 2 * P], BF16, kind="ExternalInput")
    d2_out = nc.dram_tensor("d2", [P, M], F32, kind="ExternalOutput")
    i2_out = nc.dram_tensor("i2", [P, M], U32, kind="ExternalOutput")

    with tile.TileContext(nc) as tc:
        with (
            tc.tile_pool(name="const", bufs=1) as const_pool,
            tc.tile_pool(name="dots", bufs=2) as dots_pool,
            tc.tile_pool(name="w2048", bufs=2) as w2048_pool,
            tc.tile_pool(name="cands", bufs=2) as cands_pool,
            tc.tile_pool(name="pdots", bufs=3, space="PSUM") as pdots_pool,
            tc.tile_pool(name="pdiff", bufs=2, space="PSUM") as pdiff_pool,
            tc.tile_pool(name="small", bufs=4) as small_pool,
            tc.tile_pool(name="res", bufs=1) as res_pool,
        ):
            # resident inputs
            xT_sb = [const_pool.tile([P, T], BF16, name=f"xT{c}", tag=f"xT{c}")
                     for c in range(KC)]
            for c in range(KC):
                nc.sync.dma_start(xT_sb[c][:], xTb[c])
            xr_sb = const_pool.tile([P, M * D], BF16, tag="xr")
            nc.sync.dma_start(xr_sb[:], xrb[:])
            id_sb = const_pool.tile([P, 2 * P], BF16, tag="ident")
            nc.sync.dma_start(id_sb[:], ident[:])

            # diag column ids: diagf[p, m] = 128*m + p (exact in fp32)
            diagf = const_pool.tile([P, M], F32, tag="diagf")
            nc.gpsimd.iota(diagf[:], pattern=[[P, M]], base=0,
                           channel_multiplier=1,
                           allow_small_or_imprecise_dtypes=True)
            epsb = const_pool.tile([P, 1], F32, tag="epsb")
            nc.vector.memset(epsb[:], EPS)

            d2_all = res_pool.tile([P, M], F32, tag="d2")
            i2_all = res_pool.tile([P, M], U32, tag="i2")

            dots_t = [None] * M   # bf16 dots tiles
            offs_t = [None] * M   # candidate column offsets (u32 [P,2])
            offsf_t = [None] * M  # same as f32 for the diag guard
            cand_t = [None] * M   # gathered candidate rows
            d2c_t = [None] * M    # per-candidate dist2

            def stage_a(m):
                # PE: dots for m-tile in 4 PSUM quarters; ACT: copy to bf16
                dots = dots_pool.tile([P, T], BF16, tag="dots")
                dots_t[m] = dots
                for q in range(4):
                    ps = pdots_pool.tile([P, 1024], F32, tag="ps")
                    for c in range(KC):      # c outer: weight reuse (2 mms)
                        for jj in range(2):
                            j0 = q * 1024 + jj * 512
                            nc.tensor.matmul(
                                ps[:, jj * 512:(jj + 1) * 512],
                                lhsT=xT_sb[c][:, m * P:(m + 1) * P],
                                rhs=xT_sb[c][:, j0:j0 + 512],
                                start=(c == 0), stop=(c == KC - 1))
                    nc.scalar.copy(dots[:, q * 1024:(q + 1) * 1024], ps[:])

            def stage_b(m):
                # DVE: fold-max cascade + MAX8 + FIND_INDEX8@2048
                dots = dots_t[m]
                w2048 = w2048_pool.tile([P, 2048], BF16, tag="w2048")
                nc.vector.tensor_tensor(
                    out=w2048[:], in0=dots[:, 0:2048], in1=dots[:, 2048:4096],
                    op=mybir.AluOpType.max)
                prev = w2048
                width = 1024
                folds = {}
                while width >= 128:
                    wt = small_pool.tile([P, width], BF16, tag=f"w{width}")
                    nc.vector.tensor_tensor(
                        out=wt[:], in0=prev[:, 0:width], in1=prev[:, width:2 * width],
                        op=mybir.AluOpType.max)
                    folds[width] = wt
                    prev = wt
                    width //= 2
                top8 = small_pool.tile([P, 8], BF16, tag="top8")
                nc.vector.max(out=top8[:], in_=folds[128][:])
                idx8 = small_pool.tile([P, 8], U32, tag="idx8")
                nc.vector.max_index(out=idx8[:], in_max=top8[:], in_values=w2048[:])

                # candidate columns: idx2 (clamped) and idx2 + 2048
                offs = small_pool.tile([P, 2], U32, tag="offs")
                offs_t[m] = offs
                nc.vector.tensor_scalar(
                    out=offs[:, 0:1], in0=idx8[:, 1:2], scalar1=2047,
                    scalar2=None, op0=mybir.AluOpType.min)
                nc.vector.tensor_scalar(
                    out=offs[:, 1:2], in0=offs[:, 0:1], scalar1=2048,
                    scalar2=None, op0=mybir.AluOpType.add)
                offsf = small_pool.tile([P, 2], F32, tag="offsf")
                offsf_t[m] = offsf
                nc.vector.tensor_copy(offsf[:], offs[:])
                nc.vector.tensor_copy(i2_all[:, m:m + 1], offs[:, 0:1])

            def stage_c(m):
                # DGE: gather both candidate rows from HBM (bf16)
                cands = cands_pool.tile([P, 2 * D], BF16, tag="cands")
                cand_t[m] = cands
                for k in range(2):
                    nc.gpsimd.indirect_dma_start(
                        out=cands[:, k * D:(k + 1) * D], out_offset=None,
                        in_=xgb[:],
                        in_offset=bass.IndirectOffsetOnAxis(
                            ap=offs_t[m][:, k:k + 1], axis=0))

            def stage_d(m):
                # PE: diff_k = x_t - cand_k via identity matmuls
                pd = pdiff_pool.tile([P, 2 * D], F32, tag="pd")
                d2c_t[m] = pd
                for k in range(2):
                    nc.tensor.matmul(
                        pd[:, k * D:(k + 1) * D],
                        lhsT=id_sb[:, 0:P],
                        rhs=xr_sb[:, m * D:(m + 1) * D],
                        start=True, stop=False)
                    nc.tensor.matmul(
                        pd[:, k * D:(k + 1) * D],
                        lhsT=id_sb[:, P:2 * P],
                        rhs=cand_t[m][:, k * D:(k + 1) * D],
                        start=False, stop=True)

            def stage_e(m):
                # ACT: dist2_k = sum((diff_k + eps)^2)
                pd = d2c_t[m]
                sq = small_pool.tile([P, 2 * D], BF16, tag="sq")
                d2c = small_pool.tile([P, 2], F32, tag="d2c")
                d2c_t[m] = d2c
                for k in range(2):
                    nc.scalar.activation(
                        out=sq[:, k * D:(k + 1) * D], in_=pd[:, k * D:(k + 1) * D],
                        func=mybir.ActivationFunctionType.Square,
                        bias=epsb[:], scale=1.0,
                        accum_out=d2c[:, k:k + 1])

            def stage_f(m):
                # DVE: diag guard + min over candidates
                g = small_pool.tile([P, 2], F32, tag="g")
                nc.vector.tensor_scalar(
                    out=g[:], in0=offsf_t[m][:], scalar1=diagf[:, m:m + 1],
                    scalar2=None, op0=mybir.AluOpType.is_equal)
                gd = small_pool.tile([P, 2], F32, tag="gd")
                nc.vector.scalar_tensor_tensor(
                    out=gd[:], in0=g[:], scalar=BIG, in1=d2c_t[m][:],
                    op0=mybir.AluOpType.mult, op1=mybir.AluOpType.add)
                nc.vector.tensor_tensor(
                    out=d2_all[:, m:m + 1], in0=gd[:, 0:1], in1=gd[:, 1:2],
                    op=mybir.AluOpType.min)

            for m in range(M + 2):
                if m < M:
                    stage_a(m)
                if 1 <= m <= M:
                    stage_b(m - 1)
                    stage_c(m - 1)
                    stage_d(m - 1)
                    stage_e(m - 1)
                if 2 <= m <= M + 1:
                    stage_f(m - 2)

            nc.sync.dma_start(d2_out[:], d2_all[:])
            nc.sync.dma_start(i2_out[:], i2_all[:])
    nc.compile()
    return nc


_CACHE = {}


def _built():
    if "nc" not in _CACHE:
        _CACHE["nc"] = build_bass(8)
    return _CACHE["nc"]


def make_in_maps(x):
    x = np.ascontiguousarray(np.asarray(x, dtype=np.float32))
    assert x.shape == (B, T, D)
    idm = np.eye(P, dtype=ml_dtypes.bfloat16)
    ident = np.ascontiguousarray(
        np.concatenate([idm, -idm], axis=1))  # [P, 2P]
    in_maps = []
    for b in range(B):
        xb = x[b].astype(ml_dtypes.bfloat16)
        in_maps.append({
            "xTb": np.ascontiguousarray(xb.T).reshape(KC, P, T),
            "xrb": np.ascontiguousarray(
                xb.reshape(M, P, D).transpose(1, 0, 2)).reshape(P, M * D),
            "xgb": xb,
            "ident": ident,
        })
    return in_maps


def postprocess(d2_list):
    # d2_list: per-core [128, 32] fp32 squared distances (row t = 128*m + p)
    total = 0.0
    n = 0
    for d2 in d2_list:
        d = np.sqrt(d2.astype(np.float64))
        total += np.log(d + EPS).sum()
        n += d.size
    return np.float32(-(total / n))


def kernel(student_output):
    nc = _built()
    in_maps = make_in_maps(student_output)
    res = bass_utils.run_bass_kernel_spmd(nc, in_maps, core_ids=list(range(B)))
    return postprocess([res.results[b]["d2"] for b in range(B)])


# revision 5
# speedup vs baseline: 2.3992x; 1.1192x over previous
"""KoLeo-loss kernel for Trainium2 (Bass/Tile), data-parallel over batch on 8 cores.

Input : student_output [8, 4096, 256] fp32
Output: scalar fp32 loss = -mean(log(||x - x_nn + 1e-8||_2 + 1e-8))
        where x_nn[b,t] = x[b, argmax_s <x[b,t], x[b,s]> (diag excluded)].

Per-core plan (core b handles batch b), "S7-fp8" scheme:
  - PE: gram matrix dots = x @ x.T in 32 m-tiles of [128, 4096] using
        fp8e4m3 inputs with DoubleRow perf mode: one matmul contracts
        the full K=256 (two 128-deep planes packed per PE cell) at 0.5
        cycles/row.  fp8 quantization only perturbs the *selection* of
        the argmax (ties flip to a near-equal neighbor); distances are
        computed from bf16 rows, so the loss error stays ~2e-4.
  - ACT: PSUM -> SBUF copies, downcast to bf16.
  - DVE: pairwise-max fold cascade 4096 -> 2048 -> ... -> 128 (2x mode
        in bf16), MAX8 on the 128-wide tail (top-1 is always the
        diagonal; top-2 is the NN dot), FIND_INDEX8 on the 2048-wide
        fold level.  The NN's true column is idx or idx+2048: both are
        evaluated and the row's distance is their min (flips ~0.6% of
        rows to a closer-but-lower-dot neighbor; ~2e-4 rel loss impact).
  - GPSIMD/DGE: one combined gather of both candidate rows (bf16).
  - PE: diff_k = x_t - cand_k via identity-matmul accumulation.
  - ACT: dist2_k = sum((diff_k + 1e-8)^2) via Square with accumulate.
  - host: diagonal guard + min over the 2 candidates, then
        loss = -mean(log(sqrt(dist2) + 1e-8)) in f64, over all 8 cores.
"""

import numpy as np
import ml_dtypes

import concourse.bass as bass
import concourse.tile as tile
from concourse import bacc, mybir
from concourse import bass_utils

F32 = mybir.dt.float32
BF16 = mybir.dt.bfloat16
FP8 = mybir.dt.float8e4
U32 = mybir.dt.uint32

B, T, D = 8, 4096, 256
P = 128                  # partitions
M = T // P               # 32 m-tiles
KC = D // P              # 2 contraction planes (DoubleRow)
EPS = 1e-8


def build_bass(num_devices=8):
    nc = bacc.Bacc("TRN2", target_bir_lowering=False, debug=False,
                   num_devices=num_devices)
    xT8 = nc.dram_tensor("xT8", [P, KC, T], FP8, kind="ExternalInput")
    xrb = nc.dram_tensor("xrb", [P, M * D], BF16, kind="ExternalInput")
    xgb = nc.dram_tensor("xgb", [T, D], BF16, kind="ExternalInput")
    ident = nc.dram_tensor("ident", [P, 2 * P], BF16, kind="ExternalInput")
    d2_out = nc.dram_tensor("d2", [P, 2 * M], F32, kind="ExternalOutput")
    off_out = nc.dram_tensor("off", [P, 2 * M], U32, kind="ExternalOutput")

    with tile.TileContext(nc) as tc:
        with (
            tc.tile_pool(name="const", bufs=1) as const_pool,
            tc.tile_pool(name="dots", bufs=2) as dots_pool,
            tc.tile_pool(name="w2048", bufs=2) as w2048_pool,
            tc.tile_pool(name="cands", bufs=3) as cands_pool,
            tc.tile_pool(name="pdots", bufs=3, space="PSUM") as pdots_pool,
            tc.tile_pool(name="pdiff", bufs=2, space="PSUM") as pdiff_pool,
            tc.tile_pool(name="small", bufs=4) as small_pool,
            tc.tile_pool(name="res", bufs=1) as res_pool,
        ):
            # resident inputs
            xT8_sb = const_pool.tile([P, KC, T], FP8, tag="xT8")
            nc.sync.dma_start(xT8_sb[:], xT8[:])
            xr_sb = const_pool.tile([P, M * D], BF16, tag="xr")
            nc.sync.dma_start(xr_sb[:], xrb[:])
            id_sb = const_pool.tile([P, 2 * P], BF16, tag="ident")
            nc.sync.dma_start(id_sb[:], ident[:])
            epsb = const_pool.tile([P, 1], F32, tag="epsb")
            nc.vector.memset(epsb[:], EPS)

            d2_all = res_pool.tile([P, 2 * M], F32, tag="d2")
            off_all = res_pool.tile([P, 2 * M], U32, tag="off")

            dots_t = [None] * M   # bf16 dots tiles
            cand_t = [None] * M   # gathered candidate rows
            pd_t = [None] * M     # diff PSUM tiles

            def stage_a(m):
                # PE: dots in 4 PSUM quarters (2 DoubleRow mms each);
                # ACT: copy to bf16
                dots = dots_pool.tile([P, T], BF16, tag="dots")
                dots_t[m] = dots
                lhsT = xT8_sb[:, :, m * P:(m + 1) * P]
                for q in range(4):
                    ps = pdots_pool.tile([P, 1024], F32, tag="ps")
                    for jj in range(2):
                        j0 = q * 1024 + jj * 512
                        nc.tensor.matmul(
                            ps[:, jj * 512:(jj + 1) * 512],
                            lhsT=lhsT,
                            rhs=xT8_sb[:, :, j0:j0 + 512],
                            start=True, stop=True,
                            perf_mode=mybir.MatmulPerfMode.DoubleRow)
                    nc.scalar.copy(dots[:, q * 1024:(q + 1) * 1024], ps[:])

            def stage_b(m):
                # DVE: fold-max cascade + MAX8 + FIND_INDEX8@2048 + offsets
                dots = dots_t[m]
                w2048 = w2048_pool.tile([P, 2048], BF16, tag="w2048")
                nc.vector.tensor_tensor(
                    out=w2048[:], in0=dots[:, 0:2048], in1=dots[:, 2048:4096],
                    op=mybir.AluOpType.max)
                prev = w2048
                width = 1024
                folds = {}
                while width >= 128:
                    wt = small_pool.tile([P, width], BF16, tag=f"w{width}")
                    nc.vector.tensor_tensor(
                        out=wt[:], in0=prev[:, 0:width], in1=prev[:, width:2 * width],
                        op=mybir.AluOpType.max)
                    folds[width] = wt
                    prev = wt
                    width //= 2
                top8 = small_pool.tile([P, 8], BF16, tag="top8")
                nc.vector.max(out=top8[:], in_=folds[128][:])
                idx8 = small_pool.tile([P, 8], U32, tag="idx8")
                nc.vector.max_index(out=idx8[:], in_max=top8[:], in_values=w2048[:])

                # candidate columns: idx2 (clamped) and idx2 + 2048,
                # written straight into the resident output tile
                offs = off_all[:, 2 * m:2 * m + 2]
                nc.vector.tensor_scalar(
                    out=offs[:, 0:1], in0=idx8[:, 1:2], scalar1=2047,
                    scalar2=None, op0=mybir.AluOpType.min)
                nc.vector.tensor_scalar(
                    out=offs[:, 1:2], in0=offs[:, 0:1], scalar1=2048,
                    scalar2=None, op0=mybir.AluOpType.add)

            def stage_c(m):
                # GPSIMD/DGE: gather both candidate rows in one indirect DMA
                cands = cands_pool.tile([P, 2 * D], BF16, tag="cands")
                cand_t[m] = cands
                nc.gpsimd.indirect_dma_start(
                    out=cands[:], out_offset=None,
                    in_=xgb[:],
                    in_offset=bass.IndirectOffsetOnAxis(
                        ap=off_all[:, 2 * m:2 * m + 2], axis=0))

            def stage_d(m):
                # PE: diff_k = x_t - cand_k via identity matmuls
                pd = pdiff_pool.tile([P, 2 * D], F32, tag="pd")
                pd_t[m] = pd
                for k in range(2):
                    nc.tensor.matmul(
                        pd[:, k * D:(k + 1) * D],
                        lhsT=id_sb[:, 0:P],
                        rhs=xr_sb[:, m * D:(m + 1) * D],
                        start=True, stop=False)
                    nc.tensor.matmul(
                        pd[:, k * D:(k + 1) * D],
                        lhsT=id_sb[:, P:2 * P],
                        rhs=cand_t[m][:, k * D:(k + 1) * D],
                        start=False, stop=True)

            def stage_e(m):
                # ACT: dist2_k = sum((diff_k + eps)^2) -> resident tile
                pd = pd_t[m]
                sq = small_pool.tile([P, 2 * D], BF16, tag="sq")
                for k in range(2):
                    nc.scalar.activation(
                        out=sq[:, k * D:(k + 1) * D], in_=pd[:, k * D:(k + 1) * D],
                        func=mybir.ActivationFunctionType.Square,
                        bias=epsb[:], scale=1.0,
                        accum_out=d2_all[:, 2 * m + k:2 * m + k + 1])

            for m in range(M + 2):
                if 2 <= m <= M + 1:
                    stage_d(m - 2)
                    stage_e(m - 2)
                if m < M:
                    stage_a(m)
                if 1 <= m <= M:
                    stage_b(m - 1)
                    stage_c(m - 1)

            nc.sync.dma_start(d2_out[:], d2_all[:])
            nc.sync.dma_start(off_out[:], off_all[:])
    nc.compile()
    return nc


_CACHE = {}


def _built():
    if "nc" not in _CACHE:
        _CACHE["nc"] = build_bass(8)
    return _CACHE["nc"]


def make_in_maps(x):
    x = np.ascontiguousarray(np.asarray(x, dtype=np.float32))
    assert x.shape == (B, T, D)
    idm = np.eye(P, dtype=ml_dtypes.bfloat16)
    ident = np.ascontiguousarray(np.concatenate([idm, -idm], axis=1))
    in_maps = []
    for b in range(B):
        xb = x[b].astype(ml_dtypes.bfloat16)
        x8 = x[b].astype(ml_dtypes.float8_e4m3)
        # xT8[ki, ko, t] = x8[t, ko*128 + ki]
        xT8 = np.ascontiguousarray(
            x8.T.reshape(KC, P, T).transpose(1, 0, 2))
        in_maps.append({
            "xT8": xT8,
            "xrb": np.ascontiguousarray(
                xb.reshape(M, P, D).transpose(1, 0, 2)).reshape(P, M * D),
            "xgb": xb,
            "ident": ident,
        })
    return in_maps


def postprocess(results):
    """results: per-core dicts with d2 [128, 2M] and off [128, 2M].
    Row t = 128*m + p holds candidates at columns (2m, 2m+1)."""
    total = 0.0
    n = 0
    for r in results:
        d2 = r["d2"].astype(np.float64).reshape(P, M, 2)
        off = r["off"].astype(np.int64).reshape(P, M, 2)
        rowid = np.arange(P)[:, None, None] + 128 * np.arange(M)[None, :, None]
        d2 = np.where(off == rowid, np.inf, d2).min(axis=2)  # diag guard + min
        d = np.sqrt(d2)
        total += np.log(d + EPS).sum()
        n += d.size
    return np.float32(-(total / n))


def kernel(student_output):
    nc = _built()
    in_maps = make_in_maps(student_output)
    res = bass_utils.run_bass_kernel_spmd(nc, in_maps, core_ids=list(range(B)))
    return postprocess([res.results[b] for b in range(B)])
